# revision 5
# baseline (speedup 1.0000x reference)
"""Trainium2 Bass kernel for nn_CausalMemory (reverse-causal decayed attention).

Math: out = ((qh @ xb.T) * W) @ xb @ VOB, where xb = x @ basis (rank-128),
qh = xb @ (Qc.T Kc), VOB = (Vc.T Oc) basis.T * out_scale, and
W[t,s] = decay^(s-t-1) for s>t else 0 (strictly-future attention).
decay^256 ~ 4e-6, so attention is windowed to the next J-1 chunks of 128.

Sharding: 8 cores = batch(4) x sequence-halves(2). Each core handles 2048
query tokens; its key/value range extends (J-1)*128 tokens past the query
range (zero-padded at the end of the sequence, which reproduces truncation
exactly).
"""

import numpy as np
import ml_dtypes

B, T, C, H = 4, 4096, 512, 128
TQ = 2048           # query tokens per core
CH = 128            # chunk
J = 3               # window chunks (own + J-1 ahead)
LOOK = (J - 1) * CH
TK = TQ + LOOK      # key tokens per core
NCH = TK // CH      # key chunks per core
NT = TQ // CH       # query tiles per core
BLK = 512           # token block
KBW = [min(BLK, TK - kb * BLK) for kb in range((TK + BLK - 1) // BLK)]

# packed const layouts
CB_W = 512 + 128 + 512 + 128      # basis | a_mat | vob | ident   (bf16)
CF_W = J * 128 + 512 + 1          # wmask | rq | dk               (f32)

_CACHE = {}


def _build():
    import concourse.tile as tile
    from concourse.tile_rust import add_dep_helper
    from concourse import bacc, mybir

    bf16 = mybir.dt.bfloat16
    f32 = mybir.dt.float32

    nc = bacc.Bacc("TRN2", target_bir_lowering=False, debug=False, num_devices=8)

    xt_ext = nc.declare_dram_parameter("xt", [128, 4, TK], bf16, isOutput=False)
    cb_ext = nc.declare_dram_parameter("cb", [128, CB_W], bf16, isOutput=False)
    cf_ext = nc.declare_dram_parameter("cf", [128, CF_W], f32, isOutput=False)
    out_ext = nc.declare_dram_parameter("out", [TQ, 512], bf16, isOutput=True)

    Copy = mybir.ActivationFunctionType.Copy

    with tile.TileContext(nc) as tc:
        with (
            tc.tile_pool(name="consts", bufs=1) as cpool,
            tc.tile_pool(name="xt", bufs=3) as xtp,
            tc.tile_pool(name="big", bufs=1) as bigp,
            tc.tile_pool(name="st", bufs=5) as stp,
            tc.tile_pool(name="rv", bufs=3) as rvp,
            tc.tile_pool(name="outb", bufs=2) as outp,
            tc.tile_pool(name="ps_xq", bufs=2, space="PSUM") as ps_xq,
            tc.tile_pool(name="ps_st", bufs=2, space="PSUM") as ps_stp,
            tc.tile_pool(name="ps_rv", bufs=2, space="PSUM") as ps_rvp,
            tc.tile_pool(name="ps_out", bufs=2, space="PSUM") as ps_outp,
        ):
            cb = cpool.tile([128, CB_W], bf16)
            nc.gpsimd.dma_start(cb[:], cb_ext[:])
            cf = cpool.tile([128, CF_W], f32)
            nc.gpsimd.dma_start(cf[:], cf_ext[:])
            basis_s = cb[:, 0:512]
            a_s = cb[:, 512:640]
            vob_s = cb[:, 640:1152]
            id_s = cb[:, 1152:1280]
            wm_s = cf[:, 0:J * 128]
            rq_s = cf[:, J * 128:J * 128 + 512]
            dk_s = cf[:, J * 128 + 512:J * 128 + 513]

            xb_big = bigp.tile([128, TK], bf16, tag="xb")
            xtok_big = bigp.tile([128, TK], bf16, tag="xtok")
            qh_big = bigp.tile([128, TQ], bf16, tag="qh")

            for kb, w in enumerate(KBW):
                off = kb * BLK
                xt3 = xtp.tile([128, 4, w], bf16, tag="xt")
                nc.sync.dma_start(xt3[:], xt_ext[:, :, off:off + w])
                pxb = ps_xq.tile([128, w], f32, tag="psxq")
                for sl in range(4):
                    nc.tensor.matmul(
                        pxb[:], basis_s[:, sl * 128:(sl + 1) * 128], xt3[:, sl, :],
                        start=(sl == 0), stop=(sl == 3))
                xb = xb_big[:, off:off + w]
                nc.scalar.copy(xb, pxb[:])

                ptk = ps_xq.tile([128, w], bf16, tag="psxq")
                for ci in range(w // 128):
                    nc.tensor.transpose(
                        ptk[:, ci * 128:(ci + 1) * 128],
                        xb[:, ci * 128:(ci + 1) * 128], id_s)
                nc.scalar.activation(xtok_big[:, off:off + w], ptk[:], Copy,
                                     scale=dk_s)

                if off < TQ:
                    pqh = ps_xq.tile([128, w], f32, tag="psxq")
                    nc.tensor.matmul(pqh[:], a_s, xb, start=True, stop=True)
                    nc.vector.tensor_mul(qh_big[:, off:off + w], pqh[:],
                                         rq_s[:, :w])

            # chunk-centric scores: one matmul + one masked copy per key chunk
            st_s = {}
            for c in range(NCH):
                n0 = max(0, c - (J - 1))
                n1 = min(NT - 1, c)
                L = n1 - n0 + 1
                pst = ps_stp.tile([128, J * 128], f32, tag="pst")
                nc.tensor.matmul(
                    pst[:, :L * 128],
                    xb_big[:, c * 128:(c + 1) * 128],
                    qh_big[:, n0 * 128:(n1 + 1) * 128],
                    start=True, stop=True)
                st = stp.tile([128, J * 128], bf16, tag="st")
                # slab for tile n sits at (n - n0); its j = c - n.
                # wm_s is ordered [j=J-1 | ... | j=0]; the needed slice is
                # contiguous: j runs (c-n0) down to (c-n1).
                w0 = (J - 1 - (c - n0)) * 128
                nc.vector.tensor_mul(st[:, :L * 128], pst[:, :L * 128],
                                     wm_s[:, w0:w0 + L * 128])
                st_s[c] = st

            for g in range(NT // 4):
                ob = outp.tile([128, 4, 512], bf16, tag="outb")
                for p in range(4):
                    i = g * 4 + p
                    prv = ps_rvp.tile([128, 128], f32, tag="prv")
                    for j in range(J):
                        c = i + j
                        pos = i - max(0, c - (J - 1))
                        nc.tensor.matmul(
                            prv[:],
                            xtok_big[:, c * 128:(c + 1) * 128],
                            st_s[c][:, pos * 128:(pos + 1) * 128],
                            start=(j == 0), stop=(j == J - 1))
                    rv = rvp.tile([128, 128], bf16, tag="rv")
                    nc.scalar.copy(rv[:], prv[:])

                    pout = ps_outp.tile([128, 512], f32, tag="pout")
                    nc.tensor.matmul(pout[:], rv[:], vob_s, start=True, stop=True)
                    if p % 2 == 0:
                        nc.vector.tensor_copy(ob[:, p, :], pout[:])
                    else:
                        nc.scalar.copy(ob[:, p, :], pout[:])
                nc.gpsimd.dma_start(
                    out_ext[g * 512:(g + 1) * 512, :].rearrange(
                        "(s p) c -> p s c", p=128),
                    ob[:])

    nc.compile()
    return nc


def _host_consts(basis, qc, kc, vc, oc, decay_logit, out_scale):
    bf = ml_dtypes.bfloat16
    d = 1.0 / (1.0 + np.exp(-np.float64(decay_logit)))
    basis64 = np.asarray(basis, np.float64)
    A = np.asarray(qc, np.float64).T @ np.asarray(kc, np.float64)
    VOB = (np.asarray(vc, np.float64).T @ np.asarray(oc, np.float64)) \
        @ basis64.T * np.float64(out_scale)
    # reversed slab order: slab k holds j = J-1-k  ->  [d^(128(J-1)-1) | ... | tri]
    W = np.zeros((CH, J * CH), dtype=np.float64)
    s_idx = np.arange(CH)[:, None]
    t_idx = np.arange(CH)[None, :]
    for k in range(J):
        j = J - 1 - k
        if j == 0:
            W[:, k * CH:(k + 1) * CH] = np.where(s_idx > t_idx, 1.0 / d, 0.0)
        else:
            W[:, k * CH:(k + 1) * CH] = d ** (CH * j - 1)
    rq = (d ** (-(np.arange(512, dtype=np.float64) % CH)))[None, :].repeat(128, 0)
    dk = (d ** np.arange(CH, dtype=np.float64))[:, None]

    cb = np.zeros((128, CB_W), dtype=bf)
    cb[:, 0:512] = basis64.astype(np.float32).reshape(4, 128, 128) \
        .transpose(1, 0, 2).reshape(128, 512).astype(bf)
    cb[:, 512:640] = A.astype(np.float32).astype(bf)
    cb[:, 640:1152] = VOB.astype(np.float32).astype(bf)
    cb[:, 1152:1280] = np.eye(128, dtype=np.float32).astype(bf)
    cf = np.zeros((128, CF_W), dtype=np.float32)
    cf[:, 0:J * 128] = W
    cf[:, J * 128:J * 128 + 512] = rq
    cf[:, J * 128 + 512:] = dk
    return cb, cf


def make_in_maps(x, basis, q_coeffs, k_coeffs, v_coeffs, o_coeffs,
                 decay_logit, out_scale):
    bf = ml_dtypes.bfloat16
    cb, cf = _host_consts(basis, q_coeffs, k_coeffs, v_coeffs, o_coeffs,
                          decay_logit, out_scale)
    x = np.asarray(x, np.float32)
    in_maps = []
    for b in range(B):
        xbT = np.ascontiguousarray(x[b].T)  # [C, T]
        for h in range(2):
            q0 = h * TQ
            xs = np.zeros((C, TK), dtype=np.float32)
            avail = min(TK, T - q0)
            xs[:, :avail] = xbT[:, q0:q0 + avail]
            in_maps.append({
                "xt": np.ascontiguousarray(
                    xs.reshape(4, 128, TK).transpose(1, 0, 2)).astype(bf),
                "cb": cb,
                "cf": cf,
            })
    return in_maps


def assemble_out(results):
    out = np.zeros((B, T, C), dtype=np.float32)
    for core in range(8):
        b, h = core // 2, core % 2
        out[b, h * TQ:(h + 1) * TQ, :] = np.asarray(
            results[core]["out"]).astype(np.float32)
    return out


def get_nc():
    if "nc" not in _CACHE:
        _CACHE["nc"] = _build()
    return _CACHE["nc"]


def kernel(x, basis, q_coeffs, k_coeffs, v_coeffs, o_coeffs,
           decay_logit, out_scale):
    from concourse.bass_utils import run_bass_kernel_spmd

    nc = get_nc()
    in_maps = make_in_maps(x, basis, q_coeffs, k_coeffs, v_coeffs, o_coeffs,
                           decay_logit, out_scale)
    res = run_bass_kernel_spmd(nc, in_maps, list(range(8)))
    return assemble_out(res.results)


# revision 7
# speedup vs baseline: 1.0419x; 1.0419x over previous
"""Trainium2 Bass kernel for nn_CausalMemory (reverse-causal decayed attention).

Math: out = ((qh @ xb.T) * W) @ xb @ VOB, where xb = x @ basis (rank-128),
qh = xb @ (Qc.T Kc), VOB = (Vc.T Oc) basis.T * out_scale, and
W[t,s] = decay^(s-t-1) for s>t else 0 (strictly-future attention).
decay^256 ~ 4e-6, so attention is windowed to the next J-1 chunks of 128.

Sharding: 8 cores = batch(4) x sequence-halves(2). Each core handles 2048
query tokens; its key/value range extends (J-1)*128 tokens past the query
range (zero-padded at the end of the sequence, which reproduces truncation
exactly).
"""

import numpy as np
import ml_dtypes

B, T, C, H = 4, 4096, 512, 128
TQ = 2048           # query tokens per core
CH = 128            # chunk
J = 3               # window chunks (own + J-1 ahead)
LOOK = (J - 1) * CH
TK = TQ + LOOK      # key tokens per core
NCH = TK // CH      # key chunks per core
NT = TQ // CH       # query tiles per core
BLK = 512           # token block
NKB = (TK + BLK - 1) // BLK
KBW = [min(BLK, TK - kb * BLK) for kb in range(NKB)]

CB_W = 512 + 128 + 512 + 128      # basis | a_mat | vob | ident   (bf16)
CF_W = J * 128                    # wmask                         (f32)

_CACHE = {}


def _build():
    import concourse.tile as tile
    from concourse import bacc, mybir

    bf16 = mybir.dt.bfloat16
    f32 = mybir.dt.float32

    nc = bacc.Bacc("TRN2", target_bir_lowering=False, debug=False, num_devices=8)

    xt_ext = nc.declare_dram_parameter("xt", [128, 4, TK], bf16, isOutput=False)
    cb_ext = nc.declare_dram_parameter("cb", [128, CB_W], bf16, isOutput=False)
    cf_ext = nc.declare_dram_parameter("cf", [128, CF_W], f32, isOutput=False)
    out_ext = nc.declare_dram_parameter("out", [TQ, 512], bf16, isOutput=True)

    with tile.TileContext(nc) as tc:
        with (
            tc.tile_pool(name="consts", bufs=1) as cpool,
            tc.tile_pool(name="xt", bufs=3) as xtp,
            tc.tile_pool(name="big", bufs=1) as bigp,
            tc.tile_pool(name="st", bufs=8) as stp,
            tc.tile_pool(name="rv", bufs=3) as rvp,
            tc.tile_pool(name="outb", bufs=2) as outp,
            tc.tile_pool(name="ps_xq", bufs=2, space="PSUM") as ps_xq,
            tc.tile_pool(name="ps_st", bufs=2, space="PSUM") as ps_stp,
            tc.tile_pool(name="ps_rv", bufs=2, space="PSUM") as ps_rvp,
            tc.tile_pool(name="ps_out", bufs=2, space="PSUM") as ps_outp,
        ):
            cb = cpool.tile([128, CB_W], bf16)
            nc.gpsimd.dma_start(cb[:], cb_ext[:])
            cf = cpool.tile([128, CF_W], f32)
            nc.gpsimd.dma_start(cf[:], cf_ext[:])
            basis_s = cb[:, 0:512]
            a_s = cb[:, 512:640]
            vob_s = cb[:, 640:1152]
            id_s = cb[:, 1152:1280]
            wm_s = cf[:, 0:J * 128]

            xb_big = bigp.tile([128, TK], bf16, tag="xb")
            xtok_big = bigp.tile([128, TK], bf16, tag="xtok")
            qh_big = bigp.tile([128, TQ], bf16, tag="qh")
            st_s = {}

            def block_stage(kb):
                w = KBW[kb]
                off = kb * BLK
                xt3 = xtp.tile([128, 4, w], bf16, tag="xt")
                nc.sync.dma_start(xt3[:], xt_ext[:, :, off:off + w])
                pxb = ps_xq.tile([128, w], f32, tag="psxq")
                for sl in range(4):
                    nc.tensor.matmul(
                        pxb[:], basis_s[:, sl * 128:(sl + 1) * 128], xt3[:, sl, :],
                        start=(sl == 0), stop=(sl == 3))
                xb = xb_big[:, off:off + w]
                nc.scalar.copy(xb, pxb[:])

                ptk = ps_xq.tile([128, w], bf16, tag="psxq")
                for ci in range(w // 128):
                    nc.tensor.transpose(
                        ptk[:, ci * 128:(ci + 1) * 128],
                        xb[:, ci * 128:(ci + 1) * 128], id_s)
                nc.scalar.copy(xtok_big[:, off:off + w], ptk[:])

                if off < TQ:
                    pqh = ps_xq.tile([128, w], f32, tag="psxq")
                    nc.tensor.matmul(pqh[:], a_s, xb, start=True, stop=True)
                    nc.scalar.copy(qh_big[:, off:off + w], pqh[:])

            def scores_stage(c):
                n0 = max(0, c - (J - 1))
                n1 = min(NT - 1, c)
                L = n1 - n0 + 1
                pst = ps_stp.tile([128, J * 128], f32, tag="pst")
                nc.tensor.matmul(
                    pst[:, :L * 128],
                    xb_big[:, c * 128:(c + 1) * 128],
                    qh_big[:, n0 * 128:(n1 + 1) * 128],
                    start=True, stop=True)
                st = stp.tile([128, J * 128], bf16, tag="st")
                # wm slab k holds j=J-1-k; the needed j run (c-n0 .. c-n1) is a
                # contiguous slice of it
                w0 = (J - 1 - (c - n0)) * 128
                nc.vector.tensor_mul(st[:, :L * 128], pst[:, :L * 128],
                                     wm_s[:, w0:w0 + L * 128])
                st_s[c] = st

            def out_group(g):
                ob = outp.tile([128, 4, 512], bf16, tag="outb")
                for p in range(4):
                    i = g * 4 + p
                    prv = ps_rvp.tile([128, 128], f32, tag="prv")
                    for j in range(J):
                        c = i + j
                        pos = i - max(0, c - (J - 1))
                        nc.tensor.matmul(
                            prv[:],
                            xtok_big[:, c * 128:(c + 1) * 128],
                            st_s[c][:, pos * 128:(pos + 1) * 128],
                            start=(j == 0), stop=(j == J - 1))
                    rv = rvp.tile([128, 128], bf16, tag="rv")
                    nc.vector.tensor_copy(rv[:], prv[:])

                    pout = ps_outp.tile([128, 512], f32, tag="pout")
                    nc.tensor.matmul(pout[:], rv[:], vob_s, start=True, stop=True)
                    if p == 0:
                        nc.vector.tensor_copy(ob[:, p, :], pout[:])
                    else:
                        nc.scalar.copy(ob[:, p, :], pout[:])
                nc.gpsimd.dma_start(
                    out_ext[g * 512:(g + 1) * 512, :].rearrange(
                        "(s p) c -> p s c", p=128),
                    ob[:])

            for kb in range(NKB):
                block_stage(kb)
                for c in range(4 * kb, min(4 * kb + 4, NCH)):
                    scores_stage(c)
                if kb >= 1 and kb - 1 < NT // 4:
                    out_group(kb - 1)
            for g in range(max(0, NKB - 1), NT // 4):
                out_group(g)

    nc.compile()
    return nc


def _host_consts(basis, qc, kc, vc, oc, decay_logit, out_scale):
    bf = ml_dtypes.bfloat16
    d = 1.0 / (1.0 + np.exp(-np.float64(decay_logit)))
    basis64 = np.asarray(basis, np.float64)
    A = np.asarray(qc, np.float64).T @ np.asarray(kc, np.float64)
    VOB = (np.asarray(vc, np.float64).T @ np.asarray(oc, np.float64)) \
        @ basis64.T * np.float64(out_scale)
    # full decay mask, reversed slab order: slab k holds j = J-1-k.
    # value at [s, k*128+t] = d^(128j + s - t - 1) for j>=1;  j=0: tri.
    W = np.zeros((CH, J * CH), dtype=np.float64)
    s_idx = np.arange(CH)[:, None]
    t_idx = np.arange(CH)[None, :]
    for k in range(J):
        j = J - 1 - k
        if j == 0:
            W[:, k * CH:(k + 1) * CH] = np.where(
                s_idx > t_idx, d ** np.maximum(s_idx - t_idx - 1, 0), 0.0)
        else:
            W[:, k * CH:(k + 1) * CH] = d ** (CH * j + s_idx - t_idx - 1)

    cb = np.zeros((128, CB_W), dtype=bf)
    cb[:, 0:512] = basis64.astype(np.float32).reshape(4, 128, 128) \
        .transpose(1, 0, 2).reshape(128, 512).astype(bf)
    cb[:, 512:640] = A.astype(np.float32).astype(bf)
    cb[:, 640:1152] = VOB.astype(np.float32).astype(bf)
    cb[:, 1152:1280] = np.eye(128, dtype=np.float32).astype(bf)
    cf = W.astype(np.float32)
    return cb, cf


def make_in_maps(x, basis, q_coeffs, k_coeffs, v_coeffs, o_coeffs,
                 decay_logit, out_scale):
    bf = ml_dtypes.bfloat16
    cb, cf = _host_consts(basis, q_coeffs, k_coeffs, v_coeffs, o_coeffs,
                          decay_logit, out_scale)
    x = np.asarray(x, np.float32)
    in_maps = []
    for b in range(B):
        xbT = np.ascontiguousarray(x[b].T)  # [C, T]
        for h in range(2):
            q0 = h * TQ
            xs = np.zeros((C, TK), dtype=np.float32)
            avail = min(TK, T - q0)
            xs[:, :avail] = xbT[:, q0:q0 + avail]
            in_maps.append({
                "xt": np.ascontiguousarray(
                    xs.reshape(4, 128, TK).transpose(1, 0, 2)).astype(bf),
                "cb": cb,
                "cf": cf,
            })
    return in_maps


def assemble_out(results):
    out = np.zeros((B, T, C), dtype=np.float32)
    for core in range(8):
        b, h = core // 2, core % 2
        out[b, h * TQ:(h + 1) * TQ, :] = np.asarray(
            results[core]["out"]).astype(np.float32)
    return out


def get_nc():
    if "nc" not in _CACHE:
        _CACHE["nc"] = _build()
    return _CACHE["nc"]


def kernel(x, basis, q_coeffs, k_coeffs, v_coeffs, o_coeffs,
           decay_logit, out_scale):
    from concourse.bass_utils import run_bass_kernel_spmd

    nc = get_nc()
    in_maps = make_in_maps(x, basis, q_coeffs, k_coeffs, v_coeffs, o_coeffs,
                           decay_logit, out_scale)
    res = run_bass_kernel_spmd(nc, in_maps, list(range(8)))
    return assemble_out(res.results)
